# revision 55
# baseline (speedup 1.0000x reference)
import sys

sys.path.insert(0, "/opt/trn_rl_repo")
import numpy as np
from concourse import mybir
from concourse.bass import Bass, IndirectOffsetOnAxis
from concourse import bass_utils

C = 32
H = 1024
NCORES = 8
BATCH = 16            # groups per blend batch
CELLS = 64 * 512      # pair-cells per class table per core
SLAB = 144            # local plane rows computed per core (18 coarse rows)
K = 3                 # capacity slots per (cell, xr) in the sweep
J = 32                # cell-blocks (of 128 cells) per sweep iteration
NSB = CELLS // (J * 128)   # sweep iterations per class table

_f16 = mybir.dt.float16
_i32 = mybir.dt.int32


def _build_program(gs, thr):
    """gs: tuple of 4 group counts (classA/xr0, classA/xr1, classB/xr0, classB/xr1),
    each a multiple of BATCH. thr: per-batch max table-chunk (0..7) so gathers
    can start as soon as the needed IDWT C-iteration has stored its chunk."""
    GTOT = sum(gs)
    nc = Bass()
    gA = nc.declare_dram_parameter("gA", [4, 18, 128, C], _f16, isOutput=False)
    ll2b = nc.declare_dram_parameter("ll2b", [4, 36, 256, C], _f16, isOutput=False)
    ll1b = nc.declare_dram_parameter("ll1b", [4, 72, 512, C], _f16, isOutput=False)
    idx_d = nc.declare_dram_parameter("idx", [128, GTOT], _i32, isOutput=False)
    w_d = nc.declare_dram_parameter("w", [128, 4 * GTOT], _f16, isOutput=False)
    # sweep slot weights: [p, cl, sb, j, xr, k, 4]
    WTOT = 2 * NSB * J * 2 * K * 4
    ws_d = nc.declare_dram_parameter("ws", [128, WTOT], _f16, isOutput=False)
    out_d = nc.declare_dram_parameter("out", [GTOT * 128, C], _f16, isOutput=True)
    outS_d = nc.declare_dram_parameter(
        "outS", [2 * CELLS, 2 * K * C], _f16, isOutput=True
    )

    # [k*512+u, (r,x,c)]; +256 pad rows so the sweep's p-major boundary
    # column view tbl[256:256+CELLS] stays in bounds
    cA = nc.dram_tensor("cA", [CELLS + 256, 2 * 2 * C], _f16)
    cB = nc.dram_tensor("cB", [CELLS + 256, 2 * 2 * C], _f16)
    NBATCH_TOT = sum(g // BATCH for g in gs)
    stage = nc.dram_tensor("stage", [128, NBATCH_TOT * BATCH * 256], _f16)

    dmaZ = nc.alloc_semaphore()   # pad-row zeroing stores
    evZ = nc.alloc_semaphore()    # pad-zero source buffer ready
    dmaL = nc.alloc_semaphore()   # idwt input loads
    dmaP = nc.alloc_semaphore()   # idwt stores
    evW = nc.alloc_semaphore()    # idwt compute iters
    dmaW = nc.alloc_semaphore()   # idx/w loads
    dmaG = nc.alloc_semaphore()   # indirect gathers (must start at 0)
    evB = nc.alloc_semaphore()    # blend batches
    dmaO = nc.alloc_semaphore()   # output stores
    dmaS = nc.alloc_semaphore()   # sweep table loads
    evS = nc.alloc_semaphore()    # sweep blend iterations
    dmaOS = nc.alloc_semaphore()  # sweep output stores
    dmaSG = nc.alloc_semaphore()  # gather staging stores (SBUF->DRAM)
    dmaBL = nc.alloc_semaphore()  # staged batch loads (DRAM->SBUF)
    evX = nc.alloc_semaphore()    # scalar weight expansions (per sweep slot)

    # ---- SBUF ----
    NBUF = 4                      # gather ring depth (batches in flight)
    CHW = 64 * C                  # 64 output-col-pairs worth of one parity = 2048 els
    inb = [nc.alloc_sbuf_tensor(f"in{s}", [128, 4 * CHW], _f16).ap()
           for s in range(2)]
    tmp = [nc.alloc_sbuf_tensor(f"tmp_{k}", [128, CHW], _f16).ap()
           for k in range(4)]
    # out bigbuf layout [p, w(64), rp(2), xp(2), c]
    outb = [nc.alloc_sbuf_tensor(f"out{s}", [128, 4 * CHW], _f16).ap()
            for s in range(2)]
    idx_t = nc.alloc_sbuf_tensor("idx_t", [128, GTOT], _i32).ap()
    w_t = nc.alloc_sbuf_tensor("w_t", [128, 4 * GTOT], _f16).ap()
    Gb = [nc.alloc_sbuf_tensor(f"G{s}", [128, BATCH * 256], _f16).ap()
          for s in range(NBUF)]
    Ob = [nc.alloc_sbuf_tensor(f"O{s}", [128, BATCH * C], _f16).ap() for s in range(2)]
    ws_t = nc.alloc_sbuf_tensor("ws_t", [128, WTOT], _f16).ap()
    TL = [inb[0], outb[0]]  # reuse idwt buffers (idle once tables are stored)
    UbS = nc.alloc_sbuf_tensor("UbS", [128, J * 4 * C], _f16).ap()
    U2S = nc.alloc_sbuf_tensor("U2S", [128, J * 2 * C], _f16).ap()
    ObS = [nc.alloc_sbuf_tensor(f"OS{s}", [128, J * 2 * K * C], _f16).ap()
           for s in range(2)]
    CEXP = 8                      # expanded-weight channel width (rest via
    CB = C // CEXP                # stride-0 middle-dim broadcast in the mult)
    Wexp = [nc.alloc_sbuf_tensor(f"WX{s}", [128, J * 4 * CEXP], _f16).ap()
            for s in range(2)]
    # leftover-blend scratch aliases into sweep scratch (same engine, in-order)
    Ub = UbS[:, : BATCH * 4 * C]
    U2 = U2S[:, : BATCH * 2 * C]

    # ---- IDWT ----
    # levels: A: 18 coarse rows,128 wide -> ll2 (36,256); 2 col chunks of 64
    #         B: 36 rows,256 wide -> ll1 (72,512); 4 chunks
    #         C: 72 rows,512 wide -> cA/cB tables; 8 chunks
    iters = [("A", cb) for cb in range(2)] + [("B", cb) for cb in range(4)] + \
            [("C", cb) for cb in range(8)]
    NIT = len(iters)
    NPL = {"A": 18, "B": 36, "C": 72}
    ll2v = ll2b[0].rearrange("(p two) (w xp) c -> p two w xp c", two=2, xp=2)
    ll1v = ll1b[0].rearrange("(p two) (w xp) c -> p two w xp c", two=2, xp=2)
    cAv = cA[0:CELLS].rearrange("(k u) (r x c) -> k u r x c", u=512, r=2, x=2)
    cBv = cB[0:CELLS].rearrange("(k u) (r x c) -> k u r x c", u=512, r=2, x=2)

    def src_packed(level, cb):
        cs = slice(cb * 64, cb * 64 + 64)
        if level == "A":
            return gA[:, :, cs, :].rearrange("b p w c -> p b w c")
        if level == "B":
            return ll2b[:, :, cs, :].rearrange("b p w c -> p b w c")
        return ll1b[:, :, cs, :].rearrange("b p w c -> p b w c")

    stores_per_iter = {"A": 4, "B": 4, "C": 3}
    cum_stores = []
    tot = 0
    for lv, _ in iters:
        cum_stores.append(tot)
        tot += stores_per_iter[lv] * 16
    TOT_STORES = tot
    lvlB_start, lvlC_start = 2, 6

    # zero the 256 pad rows of each table: the sweep's sb=7 boundary column
    # reads them (weight 0), and uninitialized DRAM can hold NaN (0*NaN=NaN)
    nc.vector.memset(tmp[0][:, :256], 0.0)
    nc.vector.memset(tmp[1][:, :256], 0.0)
    _i = nc.vector.tensor_tensor(
        out=tmp[0][:, :256], in0=tmp[0][:, :256], in1=tmp[1][:, :256],
        op=mybir.AluOpType.mult,
    )
    _i.then_inc(evZ, 1)
    nc.sync.wait_ge(evZ, 1)
    for _tb in (cA, cB):
        nc.sync.dma_start(
            out=_tb[CELLS : CELLS + 256].rearrange(
                "(a b) e -> a (b e)", a=128, b=2
            ),
            in_=tmp[0][:, :256],
        ).then_inc(dmaZ, 16)

    nL = 0
    nW = 0
    for i, (level, cb) in enumerate(iters):
        s = i % 2
        NP = NPL[level]
        if i >= 2:
            nc.sync.wait_ge(evW, i - 1)
        if i == lvlB_start:
            nc.sync.wait_ge(dmaP, cum_stores[lvlB_start])
        if i == lvlC_start:
            nc.sync.wait_ge(dmaP, cum_stores[lvlC_start])
        nc.sync.dma_start(
            out=inb[s][:NP].rearrange("p (b w c) -> p b w c", b=4, c=C),
            in_=src_packed(level, cb),
        ).then_inc(dmaL, 16)
        nL += 16
        nc.vector.wait_ge(dmaL, nL)
        if i >= 2:
            nc.vector.wait_ge(dmaP, cum_stores[i - 1])  # stores of iter i-2 done
        A = mybir.AluOpType.add
        S = mybir.AluOpType.subtract
        inv = inb[s][:NP].rearrange("p (b e) -> p b e", b=4)
        ll, lh, hl, hh = (inv[:, k] for k in range(4))
        t1, t2, t3, t4 = (b[:NP] for b in tmp)
        ov = outb[s][:NP].rearrange("p (w rp xp c) -> p w rp xp c", rp=2, xp=2, c=C)
        oEE = ov[:, :, 0, 0, :]
        oEO = ov[:, :, 0, 1, :]
        oOE = ov[:, :, 1, 0, :]
        oOO = ov[:, :, 1, 1, :]
        nc.vector.tensor_tensor(out=t1, in0=ll, in1=lh, op=S)   # row-even lo
        nc.vector.tensor_tensor(out=t2, in0=ll, in1=lh, op=A)   # row-odd lo
        nc.vector.tensor_tensor(out=t3, in0=hl, in1=hh, op=S)   # row-even hi
        nc.vector.tensor_tensor(out=t4, in0=hl, in1=hh, op=A)   # row-odd hi
        nc.vector.tensor_tensor(out=oEE, in0=t1, in1=t3, op=S)  # (2r, 2w)
        nc.vector.tensor_tensor(out=oEO, in0=t1, in1=t3, op=A)  # (2r, 2w+1)
        nc.vector.tensor_tensor(out=oOE, in0=t2, in1=t4, op=S)  # (2r+1, 2w)
        nc.vector.tensor_tensor(out=oOO, in0=t2, in1=t4, op=A).then_inc(evW, 1)
        nW += 1
        nc.scalar.wait_ge(evW, nW)
        ws = slice(cb * 64, cb * 64 + 64)
        bufv = outb[s][:].rearrange("p (w rp xp c) -> p w rp xp c", rp=2, xp=2, c=C)
        if level in ("A", "B"):
            dstv = ll2v if level == "A" else ll1v
            for rp in range(2):
                for xp in range(2):
                    nc.scalar.dma_start(
                        out=dstv[:NP, rp, ws, xp, :],
                        in_=bufv[:NP, :, rp, xp, :],
                    ).then_inc(dmaP, 16)
        else:
            # classA: pair k=p rows (2p, 2p+1): full cells, contiguous
            nc.scalar.dma_start(
                out=cAv[0:64, ws, :, :, :], in_=bufv[0:64]
            ).then_inc(dmaP, 16)
            # classB r0 = odd rows (2p+1): k=p
            nc.scalar.dma_start(
                out=cBv[0:64, ws, 0, :, :], in_=bufv[0:64, :, 1, :, :]
            ).then_inc(dmaP, 16)
            # classB r1 = even rows (2p), p=1..64 -> k=p-1
            nc.scalar.dma_start(
                out=cBv[0:64, ws, 1, :, :], in_=bufv[1:65, :, 0, :, :]
            ).then_inc(dmaP, 16)

    # ---- sweep: K slots per (cell, xr), plain DMA loads, static-AP blends ----
    nc.sync.dma_start(out=idx_t[:], in_=idx_d[:]).then_inc(dmaW, 16)
    nc.sync.dma_start(out=w_t[:], in_=w_d[:]).then_inc(dmaW, 16)
    nc.sync.dma_start(out=ws_t[:], in_=ws_d[:]).then_inc(dmaW, 16)
    M = mybir.AluOpType.mult
    A = mybir.AluOpType.add
    wsv = ws_t[:].rearrange(
        "p (cl sb j xr k q) -> p cl sb j xr k q", cl=2, sb=NSB, j=J, xr=2, k=K
    )
    # p-major: outS row = ((cl*NSB+sb)*128+p)*J+j -> contiguous per-partition
    # stores (128 descriptors instead of 4096)
    outSv = outS_d[:].rearrange(
        "(cl sb p j) w -> cl sb p j w", cl=2, sb=NSB, j=J, p=128
    )
    nc.scalar.wait_ge(dmaW, 48)           # ws ready before weight expansions
    nc.vector.wait_ge(dmaW, 48)
    # gpsimd: issue all leftover gathers up front (paced by evB vs blends);
    # each batch waits only for the C-iteration covering its cells' u-chunk
    nc.gpsimd.wait_ge(dmaW, 48)
    nc.gpsimd.wait_ge(dmaZ, 32)

    def chunk_stores(q):
        gi = lvlC_start + q + 1
        return TOT_STORES if gi >= NIT else cum_stores[gi]
    wv4 = w_t[:].rearrange("p (g r x) -> p g r x", r=2, x=2)
    outv = out_d[:].rearrange("(p g) c -> p g c", g=GTOT)
    batches = []
    nG = 0
    bglob = 0
    goff = 0
    for si, (tblx, xr) in enumerate([(cA, 0), (cA, 1), (cB, 0), (cB, 1)]):
        GS = gs[si]
        elen = 128 if xr == 0 else 256
        for b in range(GS // BATCH):
            s = bglob % NBUF
            nc.gpsimd.wait_ge(dmaP, chunk_stores(thr[bglob]))
            if bglob >= NBUF:
                # recycle Gb once its blend has consumed it
                nc.gpsimd.wait_ge(evB, bglob - (NBUF - 1))
            for gi in range(BATCH):
                g = goff + b * BATCH + gi
                nc.gpsimd.indirect_dma_start(
                    out=Gb[s][:, gi * 256 : gi * 256 + elen],
                    out_offset=None,
                    in_=tblx[:],
                    in_offset=IndirectOffsetOnAxis(
                        ap=idx_t[:, g : g + 1], axis=0
                    ),
                ).then_inc(dmaG, 16)
                nG += 16
            batches.append((bglob, s, goff + b * BATCH, xr, nG))
            bglob += 1
        goff += GS
    NBATCH = bglob
    nO = [0]
    stagev = stage[:].rearrange("p (b e) -> p b e", e=BATCH * 256)

    def emit_blend(entry):
        bg, s, gb0, xr, nGb = entry
        nc.vector.wait_ge(dmaG, nGb)
        if bg >= 2:
            nc.vector.wait_ge(dmaO, 16 * (bg - 1))
        gsl = slice(gb0, gb0 + BATCH)
        Gv = Gb[s][:].rearrange(
            "p (g cl r x c) -> p g cl r x c", cl=2, r=2, x=2, c=C
        )
        U2v = U2[:].rearrange("p (g r c) -> p g r c", r=2, c=C)
        Ov = Ob[bg % 2][:].rearrange("p (g c) -> p g c", c=C)
        if xr == 0:
            Uv = Ub[:].rearrange("p (g r x c) -> p g r x c", r=2, x=2, c=C)
            Wb = (
                wv4[:, gsl, :, :]
                .unsqueeze(-1)
                .broadcast_to([128, BATCH, 2, 2, C])
            )
            nc.vector.tensor_tensor(out=Uv, in0=Gv[:, :, 0], in1=Wb, op=M)
            nc.vector.tensor_tensor(
                out=U2v, in0=Uv[:, :, :, 0, :], in1=Uv[:, :, :, 1, :], op=A
            )
        else:
            GvA = Gv[:, :, 0, :, 1, :]
            GvB = Gv[:, :, 1, :, 0, :]
            WA = wv4[:, gsl, :, 0].unsqueeze(-1).broadcast_to([128, BATCH, 2, C])
            WB = wv4[:, gsl, :, 1].unsqueeze(-1).broadcast_to([128, BATCH, 2, C])
            Uv4 = Ub[:].rearrange("p (g r c) -> p g r c", r=4, c=C)
            T1 = Uv4[:, :, 0:2, :]
            T2 = Uv4[:, :, 2:4, :]
            nc.vector.tensor_tensor(out=T1, in0=GvA, in1=WA, op=M)
            nc.vector.tensor_tensor(out=T2, in0=GvB, in1=WB, op=M)
            nc.vector.tensor_tensor(out=U2v, in0=T1, in1=T2, op=A)
        nc.vector.tensor_tensor(
            out=Ov, in0=U2v[:, :, 0, :], in1=U2v[:, :, 1, :], op=A
        ).then_inc(evB, 1)
        nc.sync.wait_ge(evB, bg + 1)
        nc.sync.dma_start(out=outv[:, gsl, :], in_=Ob[bg % 2][:]).then_inc(dmaO, 16)
        nO[0] += 16

    # ---- sweep with interleaved leftover blends ----
    # evS counts completed SLOTS (6 per sweep iteration)
    nOS = 0
    emitted = 0
    staged = 0
    NSW = 2 * NSB

    nS = [0]

    def tl_load(t):
        # p-major tile: partition p holds cells p*256 + [sb*J, sb*J+J], the
        # 33rd column being each cell-run's x-neighbor (contiguous in DRAM)
        cl, sb = t // NSB, t % NSB
        tbl = cA if cl == 0 else cB
        tq = tbl[0:CELLS].rearrange("(p q) e -> p q e", q=256)
        TLv = TL[t % 2][:, : (J + 1) * 128].rearrange(
            "p (j e) -> p j e", e=128
        )
        if t >= 2:
            nc.sync.wait_ge(evS, 6 * (t - 1))
        if sb < NSB - 1:
            nc.sync.dma_start(
                out=TLv, in_=tq[:, sb * J : sb * J + J + 1]
            ).then_inc(dmaS, 16)
            nS[0] += 16
        else:
            nc.sync.dma_start(
                out=TLv[:, :J], in_=tq[:, sb * J : sb * J + J]
            ).then_inc(dmaS, 16)
            bnd = tbl[256 : 256 + CELLS].rearrange("(p q) e -> p q e", q=256)
            nc.sync.wait_ge(dmaZ, 32)
            nc.sync.dma_start(
                out=TLv[:, J : J + 1], in_=bnd[:, 0:1]
            ).then_inc(dmaS, 16)
            nS[0] += 32
        return nS[0]

    nc.sync.wait_ge(dmaP, TOT_STORES)   # tables complete before sweep loads
    tl_needed = {}
    tl_needed[0] = tl_load(0)
    tl_needed[1] = tl_load(1)
    for t in range(NSW):
        cl, sb = t // NSB, t % NSB
        s2 = t % 2
        # scalar: expand this iteration's 6 slot-weight sets over the channel
        # dim so every DVE multiply gets stride-1 operands (2x mode)
        for sig, (xr, k) in enumerate((x, kk) for x in range(2) for kk in range(K)):
            S = 6 * t + sig
            if S >= 2:
                nc.scalar.wait_ge(evS, S - 1)
            Wk = wsv[:, cl, sb, :, xr, k, :]
            nc.scalar.copy(
                out=Wexp[S % 2][:].rearrange("p (j q c) -> p j q c", q=4, c=CEXP),
                in_=Wk.unsqueeze(-1).broadcast_to([128, J, 4, CEXP]),
            ).then_inc(evX, 1)
        nc.vector.wait_ge(dmaS, tl_needed[t])
        if t >= 2:
            nc.vector.wait_ge(dmaOS, 16 * (t - 1))
        TLx = TL[s2][:, : (J + 1) * 128].rearrange(
            "p (j r x c) -> p j r x c", r=2, x=2, c=C
        )
        OSv = ObS[s2][:].rearrange("p (j xr k c) -> p j xr k c", xr=2, k=K, c=C)
        UvS = UbS[:].rearrange("p (j r x c) -> p j r x c", r=2, x=2, c=C)
        Uv4S = UbS[:].rearrange("p (j r c) -> p j r c", r=4, c=C)
        U2vS = U2S[:].rearrange("p (j r c) -> p j r c", r=2, c=C)
        # q folds (r,x); b folds the broadcast channel blocks — keeps every
        # multiply operand at 4 free dims (more dims cost DVE loop overhead)
        TLq = TL[s2][:, : (J + 1) * 128].rearrange(
            "p (j q b c) -> p j q b c", q=4, b=CB, c=CEXP
        )
        UvSq = UbS[:].rearrange("p (j q b c) -> p j q b c", q=4, b=CB, c=CEXP)
        for sig, (xr, k) in enumerate((x, kk) for x in range(2) for kk in range(K)):
            S = 6 * t + sig
            nc.vector.wait_ge(evX, S + 1)
            WexpQ = Wexp[S % 2][:].rearrange(
                "p (j q c) -> p j q c", q=4, c=CEXP
            )
            if xr == 0:
                nc.vector.tensor_tensor(
                    out=UvSq,
                    in0=TLq[:, 0:J],
                    in1=WexpQ.unsqueeze(3).broadcast_to([128, J, 4, CB, CEXP]),
                    op=M,
                )
                nc.vector.tensor_tensor(
                    out=U2vS, in0=UvS[:, :, :, 0, :], in1=UvS[:, :, :, 1, :],
                    op=A,
                )
            else:
                WexpX = Wexp[S % 2][:].rearrange(
                    "p (j r x c) -> p j r x c", r=2, x=2, c=CEXP
                )
                nc.vector.tensor_tensor(
                    out=Uv4S[:, :, 0:2, :].rearrange(
                        "p j r (b c) -> p j r b c", b=CB, c=CEXP
                    ),
                    in0=TLx[:, 0:J, :, 1, :].rearrange(
                        "p j r (b c) -> p j r b c", b=CB, c=CEXP
                    ),
                    in1=WexpX[:, :, :, 0, :].unsqueeze(3).broadcast_to(
                        [128, J, 2, CB, CEXP]
                    ),
                    op=M,
                )
                nc.vector.tensor_tensor(
                    out=Uv4S[:, :, 2:4, :].rearrange(
                        "p j r (b c) -> p j r b c", b=CB, c=CEXP
                    ),
                    in0=TLx[:, 1 : J + 1, :, 0, :].rearrange(
                        "p j r (b c) -> p j r b c", b=CB, c=CEXP
                    ),
                    in1=WexpX[:, :, :, 1, :].unsqueeze(3).broadcast_to(
                        [128, J, 2, CB, CEXP]
                    ),
                    op=M,
                )
                nc.vector.tensor_tensor(
                    out=U2vS, in0=Uv4S[:, :, 0:2, :], in1=Uv4S[:, :, 2:4, :],
                    op=A,
                )
            nc.vector.tensor_tensor(
                out=OSv[:, :, xr, k, :], in0=U2vS[:, :, 0, :],
                in1=U2vS[:, :, 1, :], op=A,
            ).then_inc(evS, 1)
        if t + 2 < NSW:
            tl_needed[t + 2] = tl_load(t + 2)
        nc.sync.wait_ge(evS, 6 * (t + 1))
        nc.sync.dma_start(out=outSv[cl, sb], in_=ObS[s2][:]).then_inc(dmaOS, 16)
        nOS += 16
        # interleave leftover blends behind the sweep, lagging the measured
        # gather completion rate (~1 batch per 27.5us vs ~30us sweep iters);
        # emitting faster than gathers complete stalls the vector engine
        # mid-sweep, which serializes the sweep itself
        target = min(NBATCH, max(0, (11 * (t - 1)) // 12))
        while emitted < target:
            emit_blend(batches[emitted])
            emitted += 1
    while emitted < NBATCH:
        emit_blend(batches[emitted])
        emitted += 1
    nc.sync.wait_ge(dmaOS, nOS)
    nc.sync.wait_ge(dmaO, nO[0])
    return nc


_NC_CACHE = {}


def _get_nc(gs, thr):
    if (gs, thr) not in _NC_CACHE:
        _NC_CACHE[(gs, thr)] = _build_program(gs, thr)
    return _NC_CACHE[(gs, thr)]


def _prep_host(pts, g0, g1, g2, g3):
    f = np.float32
    g0s = np.ascontiguousarray(g0[0].transpose(1, 2, 0)) * f(0.125)
    g3s = np.ascontiguousarray(g3[0].transpose(1, 2, 3, 0)) * f(0.6 * 0.125)
    g2s = np.ascontiguousarray(g2[0].transpose(1, 2, 3, 0)) * f(0.4 * 0.25)
    g1s = np.ascontiguousarray(g1[0].transpose(1, 2, 3, 0)) * f(0.2 * 0.5)

    n = pts.shape[0]
    W1 = f(H - 1)
    x = np.clip((pts[:, 0] + f(1.0)) * f(0.5) * W1, f(0.0), W1)
    y = np.clip((pts[:, 1] + f(1.0)) * f(0.5) * W1, f(0.0), W1)
    x0 = np.floor(x)
    y0 = np.floor(y)
    wx = (x - x0).astype(f)
    wy = (y - y0).astype(f)
    x0i = x0.astype(np.int64)
    y0i = y0.astype(np.int64)
    sx = x0i == H - 1
    x0i = np.where(sx, x0i - 1, x0i)
    wx = np.where(sx, f(1.0), wx)
    sy = y0i == H - 1
    y0i = np.where(sy, y0i - 1, y0i)
    wy = np.where(sy, f(1.0), wy)

    core = (y0i >> 7).astype(np.int32)
    yl = (y0i & 127).astype(np.int32)
    P = yl & 1
    k = yl >> 1
    u = (x0i >> 1).astype(np.int32)
    xr = (x0i & 1).astype(np.int32)
    cell = k * 512 + u
    stream = P * 2 + xr
    w4 = np.stack(
        [(1 - wy) * (1 - wx), (1 - wy) * wx, wy * (1 - wx), wy * wx], axis=1
    ).astype(np.float16)

    # chunk of the last table row a point's gather touches (xr=1 reads cell+1)
    gchunk = (np.minimum((cell & 511) + xr, 511) >> 6).astype(np.int32)
    order = np.lexsort((cell, gchunk, stream, core))
    cell_s = cell[order]
    stream_s = stream[order]
    core_s = core[order]
    w4_s = w4[order]
    P_s = stream_s >> 1
    xr_s = stream_s & 1

    # rank within (core, P, cell, xr) bin; first K go to sweep slots
    nn = cell_s.shape[0]
    binid = (((core_s.astype(np.int64) * 2 + P_s) * CELLS + cell_s) * 2 + xr_s)
    newb = np.empty(nn, bool)
    newb[0] = True
    newb[1:] = binid[1:] != binid[:-1]
    first = np.maximum.accumulate(np.where(newb, np.arange(nn), 0))
    rank = (np.arange(nn) - first).astype(np.int32)
    slot = rank < K

    # per (core, stream) leftover counts
    counts = np.zeros((NCORES, 4), np.int64)
    for c in range(NCORES):
        mc = (core_s == c) & ~slot
        for s in range(4):
            counts[c, s] = int(np.sum(mc & (stream_s == s)))
    # SPMD: shared group counts per stream = max over cores, batch-rounded
    gs = tuple(
        max(BATCH,
            int(-(-int(counts[:, s].max()) // (128 * BATCH)) * BATCH))
        for s in range(4)
    )
    GTOT = sum(gs)

    # coefficient slabs per core (zero-padded beyond grid)
    def slab(arr, r0, nr, full):
        if arr.ndim == 4:
            out = np.zeros((3, nr) + arr.shape[2:], np.float16)
            hi = min(full, r0 + nr)
            out[:, : hi - r0] = arr[:, r0:hi].astype(np.float16)
        else:
            out = np.zeros((nr,) + arr.shape[1:], np.float16)
            hi = min(full, r0 + nr)
            out[: hi - r0] = arr[r0:hi].astype(np.float16)
        return out

    thr_l = [0] * (sum(gs) // BATCH)
    in_maps = []
    for c in range(NCORES):
        idx2 = np.zeros((128, GTOT), np.int32)
        wt = np.zeros((128, GTOT, 4), np.float16)
        # sweep slot weights [p, cl, sb, j, xr, k, 4]
        wS = np.zeros((128, 2, NSB, J, 2, K, 4), np.float16)
        msl = (core_s == c) & slot
        ce = cell_s[msl]
        # p-major sweep tiles: cell = p*256 + sb*J + j
        wS[ce >> 8, P_s[msl], (ce >> 5) & (NSB - 1), ce & (J - 1), xr_s[msl],
           rank[msl]] = w4_s[msl]
        goff = 0
        for s in range(4):
            sel = (core_s == c) & (stream_s == s) & ~slot
            cells_cs = cell_s[sel]
            w_cs = w4_s[sel]
            cnt = cells_cs.shape[0]
            cap = gs[s] * 128
            assert cnt <= cap, f"stream overflow core {c} stream {s}"
            # point j -> group goff + j//128, partition j%128
            gidx = goff + np.arange(cnt) // 128
            pidx = np.arange(cnt) % 128
            idx2[pidx, gidx] = cells_cs
            wt[pidx, gidx] = w_cs
            # per-batch gather eligibility: max u-chunk (incl. xr next-cell)
            ch = np.minimum((cells_cs & 511) + s % 2, 511) >> 6
            for bb in range(gs[s] // BATCH):
                lo, hi = bb * BATCH * 128, (bb + 1) * BATCH * 128
                seg = ch[lo:min(hi, cnt)]
                thr_l[goff // BATCH + bb] = max(
                    thr_l[goff // BATCH + bb],
                    int(seg.max()) if seg.size else 0,
                )
            goff += gs[s]
        in_maps.append(
            {
                "gA": np.concatenate(
                    [slab(g0s, 16 * c, 18, 128)[None], slab(g3s, 16 * c, 18, 128)]
                ),
                "ll2b": np.concatenate(
                    [np.zeros((1, 36, 256, C), np.float16),
                     slab(g2s, 32 * c, 36, 256)]
                ),
                "ll1b": np.concatenate(
                    [np.zeros((1, 72, 512, C), np.float16),
                     slab(g1s, 64 * c, 72, 512)]
                ),
                "idx": idx2,
                "w": np.ascontiguousarray(wt.reshape(128, 4 * GTOT)),
                "ws": np.ascontiguousarray(wS.reshape(128, -1)),
            }
        )
    return in_maps, order, counts, gs, tuple(thr_l), n, (
        core_s, P_s, xr_s, cell_s, rank, slot)


def kernel(pts, g0, g1, g2, g3, _res_hook=None):
    pts = np.asarray(pts, np.float32)
    in_maps, order, counts, gs, thr, n, meta = _prep_host(
        pts, np.asarray(g0, np.float32), np.asarray(g1, np.float32),
        np.asarray(g2, np.float32), np.asarray(g3, np.float32),
    )
    core_s, P_s, xr_s, cell_s, rank, slot = meta
    nc = _get_nc(gs, thr)
    res = bass_utils.run_bass_kernel_spmd(nc, in_maps, list(range(NCORES)))
    if _res_hook is not None:
        _res_hook(res)
    out_sorted = np.empty((n, C), np.float32)
    GTOT = sum(gs)
    for c in range(NCORES):
        mc = core_s == c
        # sweep-slotted points (outS rows are (cl, sb, p, j) p-major)
        oS = res.results[c]["outS"].reshape(2, NSB, 128, J, 2, K, C)
        msl = mc & slot
        ce = cell_s[msl]
        out_sorted[msl] = oS[
            P_s[msl], (ce >> 5) & (NSB - 1), ce >> 8, ce & (J - 1),
            xr_s[msl], rank[msl]
        ].astype(np.float32)
        # leftover points, packed per stream in sorted order ((p, g) rows)
        o = res.results[c]["out"].reshape(128, GTOT, C)
        goff = 0
        for s in range(4):
            sel = mc & ~slot & ((P_s * 2 + xr_s) == s)
            cnt = int(counts[c, s])
            oo = o[:, goff : goff + gs[s]].transpose(1, 0, 2).reshape(-1, C)
            out_sorted[sel] = oo[:cnt].astype(np.float32)
            goff += gs[s]
    full = np.empty_like(out_sorted)
    full[order] = out_sorted
    return full



# revision 58
# speedup vs baseline: 1.0176x; 1.0176x over previous
import sys

sys.path.insert(0, "/opt/trn_rl_repo")
import numpy as np
from concourse import mybir
from concourse.bass import Bass, IndirectOffsetOnAxis
from concourse import bass_utils

C = 32
H = 1024
NCORES = 8
BATCH = 16            # groups per blend batch
CELLS = 64 * 512      # pair-cells per class table per core
SLAB = 144            # local plane rows computed per core (18 coarse rows)
K = 3                 # capacity slots per (cell, xr) in the sweep
J = 32                # cell-blocks (of 128 cells) per sweep iteration
NSB = CELLS // (J * 128)   # sweep iterations per class table

_f16 = mybir.dt.float16
_i32 = mybir.dt.int32


def _build_program(gs):
    """gs: tuple of 4 group counts (classA/xr0, classA/xr1, classB/xr0, classB/xr1),
    each a multiple of BATCH. One SPMD program; per-core data via in_maps."""
    GTOT = sum(gs)
    nc = Bass()
    gA = nc.declare_dram_parameter("gA", [4, 18, 128, C], _f16, isOutput=False)
    ll2b = nc.declare_dram_parameter("ll2b", [4, 36, 256, C], _f16, isOutput=False)
    ll1b = nc.declare_dram_parameter("ll1b", [4, 72, 512, C], _f16, isOutput=False)
    idx_d = nc.declare_dram_parameter("idx", [128, GTOT], _i32, isOutput=False)
    w_d = nc.declare_dram_parameter("w", [128, 4 * GTOT], _f16, isOutput=False)
    # sweep slot weights: [p, cl, sb, j, xr, k, 4]
    WTOT = 2 * NSB * J * 2 * K * 4
    ws_d = nc.declare_dram_parameter("ws", [128, WTOT], _f16, isOutput=False)
    out_d = nc.declare_dram_parameter("out", [GTOT * 128, C], _f16, isOutput=True)
    outS_d = nc.declare_dram_parameter(
        "outS", [2 * CELLS, 2 * K * C], _f16, isOutput=True
    )

    # [k*512+u, (r,x,c)]; +256 pad rows so the sweep's p-major boundary
    # column view tbl[256:256+CELLS] stays in bounds
    cA = nc.dram_tensor("cA", [CELLS + 256, 2 * 2 * C], _f16)
    cB = nc.dram_tensor("cB", [CELLS + 256, 2 * 2 * C], _f16)
    NBATCH_TOT = sum(g // BATCH for g in gs)
    stage = nc.dram_tensor("stage", [128, NBATCH_TOT * BATCH * 256], _f16)

    dmaZ = nc.alloc_semaphore()   # pad-row zeroing stores
    evZ = nc.alloc_semaphore()    # pad-zero source buffer ready
    dmaL = nc.alloc_semaphore()   # idwt input loads
    dmaP = nc.alloc_semaphore()   # idwt stores
    evW = nc.alloc_semaphore()    # idwt compute iters
    dmaW = nc.alloc_semaphore()   # idx/w loads
    dmaG = nc.alloc_semaphore()   # indirect gathers (must start at 0)
    evB = nc.alloc_semaphore()    # blend batches
    dmaO = nc.alloc_semaphore()   # output stores
    dmaS = nc.alloc_semaphore()   # sweep table loads
    evS = nc.alloc_semaphore()    # sweep blend iterations
    dmaOS = nc.alloc_semaphore()  # sweep output stores
    dmaSG = nc.alloc_semaphore()  # gather staging stores (SBUF->DRAM)
    dmaBL = nc.alloc_semaphore()  # staged batch loads (DRAM->SBUF)
    evX = nc.alloc_semaphore()    # scalar weight expansions (per sweep slot)

    # ---- SBUF ----
    NBUF = 4                      # gather ring depth (batches in flight)
    CHW = 64 * C                  # 64 output-col-pairs worth of one parity = 2048 els
    inb = [nc.alloc_sbuf_tensor(f"in{s}", [128, 4 * CHW], _f16).ap()
           for s in range(2)]
    tmp = [nc.alloc_sbuf_tensor(f"tmp_{k}", [128, CHW], _f16).ap()
           for k in range(4)]
    # out bigbuf layout [p, w(64), rp(2), xp(2), c]
    outb = [nc.alloc_sbuf_tensor(f"out{s}", [128, 4 * CHW], _f16).ap()
            for s in range(2)]
    idx_t = nc.alloc_sbuf_tensor("idx_t", [128, GTOT], _i32).ap()
    w_t = nc.alloc_sbuf_tensor("w_t", [128, 4 * GTOT], _f16).ap()
    Gb = [nc.alloc_sbuf_tensor(f"G{s}", [128, BATCH * 256], _f16).ap()
          for s in range(NBUF)]
    Ob = [nc.alloc_sbuf_tensor(f"O{s}", [128, BATCH * C], _f16).ap() for s in range(2)]
    ws_t = nc.alloc_sbuf_tensor("ws_t", [128, WTOT], _f16).ap()
    TL = [inb[0], outb[0]]  # reuse idwt buffers (idle once tables are stored)
    UbS = nc.alloc_sbuf_tensor("UbS", [128, J * 4 * C], _f16).ap()
    U2S = nc.alloc_sbuf_tensor("U2S", [128, J * 2 * C], _f16).ap()
    ObS = [nc.alloc_sbuf_tensor(f"OS{s}", [128, J * 2 * K * C], _f16).ap()
           for s in range(2)]
    CEXP = 8                      # expanded-weight channel width (rest via
    CB = C // CEXP                # stride-0 middle-dim broadcast in the mult)
    Wexp = [nc.alloc_sbuf_tensor(f"WX{s}", [128, J * 4 * CEXP], _f16).ap()
            for s in range(2)]
    # leftover-blend scratch aliases into sweep scratch (same engine, in-order)
    Ub = UbS[:, : BATCH * 4 * C]
    U2 = U2S[:, : BATCH * 2 * C]

    # ---- IDWT ----
    # levels: A: 18 coarse rows,128 wide -> ll2 (36,256); 2 col chunks of 64
    #         B: 36 rows,256 wide -> ll1 (72,512); 4 chunks
    #         C: 72 rows,512 wide -> cA/cB tables; 8 chunks
    iters = [("A", cb) for cb in range(2)] + [("B", cb) for cb in range(4)] + \
            [("C", cb) for cb in range(8)]
    NIT = len(iters)
    NPL = {"A": 18, "B": 36, "C": 72}
    ll2v = ll2b[0].rearrange("(p two) (w xp) c -> p two w xp c", two=2, xp=2)
    ll1v = ll1b[0].rearrange("(p two) (w xp) c -> p two w xp c", two=2, xp=2)
    cAv = cA[0:CELLS].rearrange("(k u) (r x c) -> k u r x c", u=512, r=2, x=2)
    cBv = cB[0:CELLS].rearrange("(k u) (r x c) -> k u r x c", u=512, r=2, x=2)

    def src_packed(level, cb):
        cs = slice(cb * 64, cb * 64 + 64)
        if level == "A":
            return gA[:, :, cs, :].rearrange("b p w c -> p b w c")
        if level == "B":
            return ll2b[:, :, cs, :].rearrange("b p w c -> p b w c")
        return ll1b[:, :, cs, :].rearrange("b p w c -> p b w c")

    stores_per_iter = {"A": 4, "B": 4, "C": 3}
    cum_stores = []
    tot = 0
    for lv, _ in iters:
        cum_stores.append(tot)
        tot += stores_per_iter[lv] * 16
    TOT_STORES = tot
    lvlB_start, lvlC_start = 2, 6

    # zero the 256 pad rows of each table: the sweep's sb=7 boundary column
    # reads them (weight 0), and uninitialized DRAM can hold NaN (0*NaN=NaN)
    nc.vector.memset(tmp[0][:, :256], 0.0)
    nc.vector.memset(tmp[1][:, :256], 0.0)
    _i = nc.vector.tensor_tensor(
        out=tmp[0][:, :256], in0=tmp[0][:, :256], in1=tmp[1][:, :256],
        op=mybir.AluOpType.mult,
    )
    _i.then_inc(evZ, 1)
    nc.sync.wait_ge(evZ, 1)
    for _tb in (cA, cB):
        nc.sync.dma_start(
            out=_tb[CELLS : CELLS + 256].rearrange(
                "(a b) e -> a (b e)", a=128, b=2
            ),
            in_=tmp[0][:, :256],
        ).then_inc(dmaZ, 16)

    nL = 0
    nW = 0
    for i, (level, cb) in enumerate(iters):
        s = i % 2
        NP = NPL[level]
        if i >= 2:
            nc.sync.wait_ge(evW, i - 1)
        if i == lvlB_start:
            nc.sync.wait_ge(dmaP, cum_stores[lvlB_start])
        if i == lvlC_start:
            nc.sync.wait_ge(dmaP, cum_stores[lvlC_start])
        nc.sync.dma_start(
            out=inb[s][:NP].rearrange("p (b w c) -> p b w c", b=4, c=C),
            in_=src_packed(level, cb),
        ).then_inc(dmaL, 16)
        nL += 16
        nc.vector.wait_ge(dmaL, nL)
        if i >= 2:
            nc.vector.wait_ge(dmaP, cum_stores[i - 1])  # stores of iter i-2 done
        A = mybir.AluOpType.add
        S = mybir.AluOpType.subtract
        inv = inb[s][:NP].rearrange("p (b e) -> p b e", b=4)
        ll, lh, hl, hh = (inv[:, k] for k in range(4))
        t1, t2, t3, t4 = (b[:NP] for b in tmp)
        ov = outb[s][:NP].rearrange("p (w rp xp c) -> p w rp xp c", rp=2, xp=2, c=C)
        oEE = ov[:, :, 0, 0, :]
        oEO = ov[:, :, 0, 1, :]
        oOE = ov[:, :, 1, 0, :]
        oOO = ov[:, :, 1, 1, :]
        nc.vector.tensor_tensor(out=t1, in0=ll, in1=lh, op=S)   # row-even lo
        nc.vector.tensor_tensor(out=t2, in0=ll, in1=lh, op=A)   # row-odd lo
        nc.vector.tensor_tensor(out=t3, in0=hl, in1=hh, op=S)   # row-even hi
        nc.vector.tensor_tensor(out=t4, in0=hl, in1=hh, op=A)   # row-odd hi
        nc.vector.tensor_tensor(out=oEE, in0=t1, in1=t3, op=S)  # (2r, 2w)
        nc.vector.tensor_tensor(out=oEO, in0=t1, in1=t3, op=A)  # (2r, 2w+1)
        nc.vector.tensor_tensor(out=oOE, in0=t2, in1=t4, op=S)  # (2r+1, 2w)
        nc.vector.tensor_tensor(out=oOO, in0=t2, in1=t4, op=A).then_inc(evW, 1)
        nW += 1
        nc.scalar.wait_ge(evW, nW)
        ws = slice(cb * 64, cb * 64 + 64)
        bufv = outb[s][:].rearrange("p (w rp xp c) -> p w rp xp c", rp=2, xp=2, c=C)
        if level in ("A", "B"):
            dstv = ll2v if level == "A" else ll1v
            for rp in range(2):
                for xp in range(2):
                    nc.scalar.dma_start(
                        out=dstv[:NP, rp, ws, xp, :],
                        in_=bufv[:NP, :, rp, xp, :],
                    ).then_inc(dmaP, 16)
        else:
            # classA: pair k=p rows (2p, 2p+1): full cells, contiguous
            nc.scalar.dma_start(
                out=cAv[0:64, ws, :, :, :], in_=bufv[0:64]
            ).then_inc(dmaP, 16)
            # classB r0 = odd rows (2p+1): k=p
            nc.scalar.dma_start(
                out=cBv[0:64, ws, 0, :, :], in_=bufv[0:64, :, 1, :, :]
            ).then_inc(dmaP, 16)
            # classB r1 = even rows (2p), p=1..64 -> k=p-1
            nc.scalar.dma_start(
                out=cBv[0:64, ws, 1, :, :], in_=bufv[1:65, :, 0, :, :]
            ).then_inc(dmaP, 16)

    # ---- sweep: K slots per (cell, xr), plain DMA loads, static-AP blends ----
    nc.sync.dma_start(out=idx_t[:], in_=idx_d[:]).then_inc(dmaW, 16)
    nc.sync.dma_start(out=w_t[:], in_=w_d[:]).then_inc(dmaW, 16)
    nc.sync.dma_start(out=ws_t[:], in_=ws_d[:]).then_inc(dmaW, 16)
    M = mybir.AluOpType.mult
    A = mybir.AluOpType.add
    wsv = ws_t[:].rearrange(
        "p (cl sb j xr k q) -> p cl sb j xr k q", cl=2, sb=NSB, j=J, xr=2, k=K
    )
    # p-major: outS row = ((cl*NSB+sb)*128+p)*J+j -> contiguous per-partition
    # stores (128 descriptors instead of 4096)
    outSv = outS_d[:].rearrange(
        "(cl sb p j) w -> cl sb p j w", cl=2, sb=NSB, j=J, p=128
    )
    nc.scalar.wait_ge(dmaW, 48)           # ws ready before weight expansions
    nc.vector.wait_ge(dmaW, 48)
    # gpsimd: issue all leftover gathers up front (paced by evB vs blends)
    nc.gpsimd.wait_ge(dmaP, TOT_STORES)
    nc.gpsimd.wait_ge(dmaW, 48)
    wv4 = w_t[:].rearrange("p (g r x) -> p g r x", r=2, x=2)
    outv = out_d[:].rearrange("(p g) c -> p g c", g=GTOT)
    batches = []
    nG = 0
    bglob = 0
    goff = 0
    for si, (tblx, xr) in enumerate([(cA, 0), (cA, 1), (cB, 0), (cB, 1)]):
        GS = gs[si]
        elen = 128 if xr == 0 else 256
        for b in range(GS // BATCH):
            s = bglob % NBUF
            if bglob >= NBUF:
                # recycle Gb once its blend has consumed it
                nc.gpsimd.wait_ge(evB, bglob - (NBUF - 1))
            for gi in range(BATCH):
                g = goff + b * BATCH + gi
                nc.gpsimd.indirect_dma_start(
                    out=Gb[s][:, gi * 256 : gi * 256 + elen],
                    out_offset=None,
                    in_=tblx[:],
                    in_offset=IndirectOffsetOnAxis(
                        ap=idx_t[:, g : g + 1], axis=0
                    ),
                ).then_inc(dmaG, 16)
                nG += 16
            batches.append((bglob, s, goff + b * BATCH, xr, nG))
            bglob += 1
        goff += GS
    NBATCH = bglob
    nO = [0]
    stagev = stage[:].rearrange("p (b e) -> p b e", e=BATCH * 256)

    def emit_blend(entry):
        bg, s, gb0, xr, nGb = entry
        nc.vector.wait_ge(dmaG, nGb)
        if bg >= 2:
            nc.vector.wait_ge(dmaO, 16 * (bg - 1))
        gsl = slice(gb0, gb0 + BATCH)
        Gv = Gb[s][:].rearrange(
            "p (g cl r x c) -> p g cl r x c", cl=2, r=2, x=2, c=C
        )
        U2v = U2[:].rearrange("p (g r c) -> p g r c", r=2, c=C)
        Ov = Ob[bg % 2][:].rearrange("p (g c) -> p g c", c=C)
        if xr == 0:
            Uv = Ub[:].rearrange("p (g r x c) -> p g r x c", r=2, x=2, c=C)
            Wb = (
                wv4[:, gsl, :, :]
                .unsqueeze(-1)
                .broadcast_to([128, BATCH, 2, 2, C])
            )
            nc.vector.tensor_tensor(out=Uv, in0=Gv[:, :, 0], in1=Wb, op=M)
            nc.vector.tensor_tensor(
                out=U2v, in0=Uv[:, :, :, 0, :], in1=Uv[:, :, :, 1, :], op=A
            )
        else:
            GvA = Gv[:, :, 0, :, 1, :]
            GvB = Gv[:, :, 1, :, 0, :]
            WA = wv4[:, gsl, :, 0].unsqueeze(-1).broadcast_to([128, BATCH, 2, C])
            WB = wv4[:, gsl, :, 1].unsqueeze(-1).broadcast_to([128, BATCH, 2, C])
            Uv4 = Ub[:].rearrange("p (g r c) -> p g r c", r=4, c=C)
            T1 = Uv4[:, :, 0:2, :]
            T2 = Uv4[:, :, 2:4, :]
            nc.vector.tensor_tensor(out=T1, in0=GvA, in1=WA, op=M)
            nc.vector.tensor_tensor(out=T2, in0=GvB, in1=WB, op=M)
            nc.vector.tensor_tensor(out=U2v, in0=T1, in1=T2, op=A)
        nc.vector.tensor_tensor(
            out=Ov, in0=U2v[:, :, 0, :], in1=U2v[:, :, 1, :], op=A
        ).then_inc(evB, 1)
        nc.sync.wait_ge(evB, bg + 1)
        nc.sync.dma_start(out=outv[:, gsl, :], in_=Ob[bg % 2][:]).then_inc(dmaO, 16)
        nO[0] += 16

    # ---- sweep with interleaved leftover blends ----
    # evS counts completed SLOTS (6 per sweep iteration)
    nOS = 0
    emitted = 0
    staged = 0
    NSW = 2 * NSB

    nS = [0]

    def tl_load(t):
        # p-major tile: partition p holds cells p*256 + [sb*J, sb*J+J], the
        # 33rd column being each cell-run's x-neighbor (contiguous in DRAM)
        cl, sb = t // NSB, t % NSB
        tbl = cA if cl == 0 else cB
        tq = tbl[0:CELLS].rearrange("(p q) e -> p q e", q=256)
        TLv = TL[t % 2][:, : (J + 1) * 128].rearrange(
            "p (j e) -> p j e", e=128
        )
        if t >= 2:
            nc.sync.wait_ge(evS, 6 * (t - 1))
        if sb < NSB - 1:
            nc.sync.dma_start(
                out=TLv, in_=tq[:, sb * J : sb * J + J + 1]
            ).then_inc(dmaS, 16)
            nS[0] += 16
        else:
            nc.sync.dma_start(
                out=TLv[:, :J], in_=tq[:, sb * J : sb * J + J]
            ).then_inc(dmaS, 16)
            bnd = tbl[256 : 256 + CELLS].rearrange("(p q) e -> p q e", q=256)
            nc.sync.wait_ge(dmaZ, 32)
            nc.sync.dma_start(
                out=TLv[:, J : J + 1], in_=bnd[:, 0:1]
            ).then_inc(dmaS, 16)
            nS[0] += 32
        return nS[0]

    nc.sync.wait_ge(dmaP, TOT_STORES)   # tables complete before sweep loads
    tl_needed = {}
    tl_needed[0] = tl_load(0)
    tl_needed[1] = tl_load(1)
    for t in range(NSW):
        cl, sb = t // NSB, t % NSB
        s2 = t % 2
        # scalar: expand this iteration's 6 slot-weight sets over the channel
        # dim so every DVE multiply gets stride-1 operands (2x mode)
        for sig, (xr, k) in enumerate((x, kk) for x in range(2) for kk in range(K)):
            S = 6 * t + sig
            if S >= 2:
                nc.scalar.wait_ge(evS, S - 1)
            Wk = wsv[:, cl, sb, :, xr, k, :]
            nc.scalar.copy(
                out=Wexp[S % 2][:].rearrange("p (j q c) -> p j q c", q=4, c=CEXP),
                in_=Wk.unsqueeze(-1).broadcast_to([128, J, 4, CEXP]),
            ).then_inc(evX, 1)
        nc.vector.wait_ge(dmaS, tl_needed[t])
        if t >= 2:
            nc.vector.wait_ge(dmaOS, 16 * (t - 1))
        TLx = TL[s2][:, : (J + 1) * 128].rearrange(
            "p (j r x c) -> p j r x c", r=2, x=2, c=C
        )
        OSv = ObS[s2][:].rearrange("p (j xr k c) -> p j xr k c", xr=2, k=K, c=C)
        UvS = UbS[:].rearrange("p (j r x c) -> p j r x c", r=2, x=2, c=C)
        Uv4S = UbS[:].rearrange("p (j r c) -> p j r c", r=4, c=C)
        U2vS = U2S[:].rearrange("p (j r c) -> p j r c", r=2, c=C)
        # q folds (r,x); b folds the broadcast channel blocks — keeps every
        # multiply operand at 4 free dims (more dims cost DVE loop overhead)
        TLq = TL[s2][:, : (J + 1) * 128].rearrange(
            "p (j q b c) -> p j q b c", q=4, b=CB, c=CEXP
        )
        UvSq = UbS[:].rearrange("p (j q b c) -> p j q b c", q=4, b=CB, c=CEXP)
        for sig, (xr, k) in enumerate((x, kk) for x in range(2) for kk in range(K)):
            S = 6 * t + sig
            nc.vector.wait_ge(evX, S + 1)
            WexpQ = Wexp[S % 2][:].rearrange(
                "p (j q c) -> p j q c", q=4, c=CEXP
            )
            if xr == 0:
                nc.vector.tensor_tensor(
                    out=UvSq,
                    in0=TLq[:, 0:J],
                    in1=WexpQ.unsqueeze(3).broadcast_to([128, J, 4, CB, CEXP]),
                    op=M,
                )
                nc.vector.tensor_tensor(
                    out=U2vS, in0=UvS[:, :, :, 0, :], in1=UvS[:, :, :, 1, :],
                    op=A,
                )
            else:
                WexpX = Wexp[S % 2][:].rearrange(
                    "p (j r x c) -> p j r x c", r=2, x=2, c=CEXP
                )
                nc.vector.tensor_tensor(
                    out=Uv4S[:, :, 0:2, :].rearrange(
                        "p j r (b c) -> p j r b c", b=CB, c=CEXP
                    ),
                    in0=TLx[:, 0:J, :, 1, :].rearrange(
                        "p j r (b c) -> p j r b c", b=CB, c=CEXP
                    ),
                    in1=WexpX[:, :, :, 0, :].unsqueeze(3).broadcast_to(
                        [128, J, 2, CB, CEXP]
                    ),
                    op=M,
                )
                nc.vector.tensor_tensor(
                    out=Uv4S[:, :, 2:4, :].rearrange(
                        "p j r (b c) -> p j r b c", b=CB, c=CEXP
                    ),
                    in0=TLx[:, 1 : J + 1, :, 0, :].rearrange(
                        "p j r (b c) -> p j r b c", b=CB, c=CEXP
                    ),
                    in1=WexpX[:, :, :, 1, :].unsqueeze(3).broadcast_to(
                        [128, J, 2, CB, CEXP]
                    ),
                    op=M,
                )
                nc.vector.tensor_tensor(
                    out=U2vS, in0=Uv4S[:, :, 0:2, :], in1=Uv4S[:, :, 2:4, :],
                    op=A,
                )
            nc.vector.tensor_tensor(
                out=OSv[:, :, xr, k, :], in0=U2vS[:, :, 0, :],
                in1=U2vS[:, :, 1, :], op=A,
            ).then_inc(evS, 1)
        if t + 2 < NSW:
            tl_needed[t + 2] = tl_load(t + 2)
        nc.sync.wait_ge(evS, 6 * (t + 1))
        nc.sync.dma_start(out=outSv[cl, sb], in_=ObS[s2][:]).then_inc(dmaOS, 16)
        nOS += 16
        # interleave leftover blends behind the sweep, lagging the measured
        # gather completion rate (~1 batch per 27.5us vs ~30us sweep iters);
        # emitting faster than gathers complete stalls the vector engine
        # mid-sweep, which serializes the sweep itself
        target = min(NBATCH, max(0, (11 * (t - 1)) // 12))
        while emitted < target:
            emit_blend(batches[emitted])
            emitted += 1
    while emitted < NBATCH:
        emit_blend(batches[emitted])
        emitted += 1
    nc.sync.wait_ge(dmaOS, nOS)
    nc.sync.wait_ge(dmaO, nO[0])
    return nc


_NC_CACHE = {}


def _get_nc(gs):
    if gs not in _NC_CACHE:
        _NC_CACHE[gs] = _build_program(gs)
    return _NC_CACHE[gs]


def _prep_host(pts, g0, g1, g2, g3):
    f = np.float32
    g0s = np.ascontiguousarray(g0[0].transpose(1, 2, 0)) * f(0.125)
    g3s = np.ascontiguousarray(g3[0].transpose(1, 2, 3, 0)) * f(0.6 * 0.125)
    g2s = np.ascontiguousarray(g2[0].transpose(1, 2, 3, 0)) * f(0.4 * 0.25)
    g1s = np.ascontiguousarray(g1[0].transpose(1, 2, 3, 0)) * f(0.2 * 0.5)

    n = pts.shape[0]
    W1 = f(H - 1)
    x = np.clip((pts[:, 0] + f(1.0)) * f(0.5) * W1, f(0.0), W1)
    y = np.clip((pts[:, 1] + f(1.0)) * f(0.5) * W1, f(0.0), W1)
    x0 = np.floor(x)
    y0 = np.floor(y)
    wx = (x - x0).astype(f)
    wy = (y - y0).astype(f)
    x0i = x0.astype(np.int64)
    y0i = y0.astype(np.int64)
    sx = x0i == H - 1
    x0i = np.where(sx, x0i - 1, x0i)
    wx = np.where(sx, f(1.0), wx)
    sy = y0i == H - 1
    y0i = np.where(sy, y0i - 1, y0i)
    wy = np.where(sy, f(1.0), wy)

    core = (y0i >> 7).astype(np.int32)
    yl = (y0i & 127).astype(np.int32)
    P = yl & 1
    k = yl >> 1
    u = (x0i >> 1).astype(np.int32)
    xr = (x0i & 1).astype(np.int32)
    cell = k * 512 + u
    stream = P * 2 + xr
    w4 = np.stack(
        [(1 - wy) * (1 - wx), (1 - wy) * wx, wy * (1 - wx), wy * wx], axis=1
    ).astype(np.float16)

    order = np.lexsort((cell, stream, core))
    cell_s = cell[order]
    stream_s = stream[order]
    core_s = core[order]
    w4_s = w4[order]
    P_s = stream_s >> 1
    xr_s = stream_s & 1

    # rank within (core, P, cell, xr) bin; first K go to sweep slots
    nn = cell_s.shape[0]
    binid = (((core_s.astype(np.int64) * 2 + P_s) * CELLS + cell_s) * 2 + xr_s)
    newb = np.empty(nn, bool)
    newb[0] = True
    newb[1:] = binid[1:] != binid[:-1]
    first = np.maximum.accumulate(np.where(newb, np.arange(nn), 0))
    rank = (np.arange(nn) - first).astype(np.int32)
    slot = rank < K

    # per (core, stream) leftover counts
    counts = np.zeros((NCORES, 4), np.int64)
    for c in range(NCORES):
        mc = (core_s == c) & ~slot
        for s in range(4):
            counts[c, s] = int(np.sum(mc & (stream_s == s)))
    # SPMD: shared group counts per stream = max over cores, batch-rounded
    gs = tuple(
        max(BATCH,
            int(-(-int(counts[:, s].max()) // (128 * BATCH)) * BATCH))
        for s in range(4)
    )
    GTOT = sum(gs)

    # coefficient slabs per core (zero-padded beyond grid)
    def slab(arr, r0, nr, full):
        if arr.ndim == 4:
            out = np.zeros((3, nr) + arr.shape[2:], np.float16)
            hi = min(full, r0 + nr)
            out[:, : hi - r0] = arr[:, r0:hi].astype(np.float16)
        else:
            out = np.zeros((nr,) + arr.shape[1:], np.float16)
            hi = min(full, r0 + nr)
            out[: hi - r0] = arr[r0:hi].astype(np.float16)
        return out

    in_maps = []
    for c in range(NCORES):
        idx2 = np.zeros((128, GTOT), np.int32)
        wt = np.zeros((128, GTOT, 4), np.float16)
        # sweep slot weights [p, cl, sb, j, xr, k, 4]
        wS = np.zeros((128, 2, NSB, J, 2, K, 4), np.float16)
        msl = (core_s == c) & slot
        ce = cell_s[msl]
        # p-major sweep tiles: cell = p*256 + sb*J + j
        wS[ce >> 8, P_s[msl], (ce >> 5) & (NSB - 1), ce & (J - 1), xr_s[msl],
           rank[msl]] = w4_s[msl]
        goff = 0
        for s in range(4):
            sel = (core_s == c) & (stream_s == s) & ~slot
            cells_cs = cell_s[sel]
            w_cs = w4_s[sel]
            cnt = cells_cs.shape[0]
            cap = gs[s] * 128
            assert cnt <= cap, f"stream overflow core {c} stream {s}"
            # point j -> group goff + j//128, partition j%128
            gidx = goff + np.arange(cnt) // 128
            pidx = np.arange(cnt) % 128
            idx2[pidx, gidx] = cells_cs
            wt[pidx, gidx] = w_cs
            goff += gs[s]
        in_maps.append(
            {
                "gA": np.concatenate(
                    [slab(g0s, 16 * c, 18, 128)[None], slab(g3s, 16 * c, 18, 128)]
                ),
                "ll2b": np.concatenate(
                    [np.zeros((1, 36, 256, C), np.float16),
                     slab(g2s, 32 * c, 36, 256)]
                ),
                "ll1b": np.concatenate(
                    [np.zeros((1, 72, 512, C), np.float16),
                     slab(g1s, 64 * c, 72, 512)]
                ),
                "idx": idx2,
                "w": np.ascontiguousarray(wt.reshape(128, 4 * GTOT)),
                "ws": np.ascontiguousarray(wS.reshape(128, -1)),
            }
        )
    return in_maps, order, counts, gs, n, (core_s, P_s, xr_s, cell_s, rank, slot)


def kernel(pts, g0, g1, g2, g3, _res_hook=None):
    pts = np.asarray(pts, np.float32)
    in_maps, order, counts, gs, n, meta = _prep_host(
        pts, np.asarray(g0, np.float32), np.asarray(g1, np.float32),
        np.asarray(g2, np.float32), np.asarray(g3, np.float32),
    )
    core_s, P_s, xr_s, cell_s, rank, slot = meta
    nc = _get_nc(gs)
    res = bass_utils.run_bass_kernel_spmd(nc, in_maps, list(range(NCORES)))
    if _res_hook is not None:
        _res_hook(res)
    out_sorted = np.empty((n, C), np.float32)
    GTOT = sum(gs)
    for c in range(NCORES):
        mc = core_s == c
        # sweep-slotted points (outS rows are (cl, sb, p, j) p-major)
        oS = res.results[c]["outS"].reshape(2, NSB, 128, J, 2, K, C)
        msl = mc & slot
        ce = cell_s[msl]
        out_sorted[msl] = oS[
            P_s[msl], (ce >> 5) & (NSB - 1), ce >> 8, ce & (J - 1),
            xr_s[msl], rank[msl]
        ].astype(np.float32)
        # leftover points, packed per stream in sorted order ((p, g) rows)
        o = res.results[c]["out"].reshape(128, GTOT, C)
        goff = 0
        for s in range(4):
            sel = mc & ~slot & ((P_s * 2 + xr_s) == s)
            cnt = int(counts[c, s])
            oo = o[:, goff : goff + gs[s]].transpose(1, 0, 2).reshape(-1, C)
            out_sorted[sel] = oo[:cnt].astype(np.float32)
            goff += gs[s]
    full = np.empty_like(out_sorted)
    full[order] = out_sorted
    return full

